# revision 1
# baseline (speedup 1.0000x reference)
"""CollapsePool kernel for Trainium2 (8 NeuronCores).

Structure:
  1. Host: exact port of the reference heap-driven greedy collapse.
     The reference builds its heap once (no pushes), so pop order is exactly
     ascending (mag, node_id): a stable argsort over per-graph mags
     reproduces it bit-for-bit, including duplicate-neighbour semantics.
  2. Device (the memory-bound bulk): gather x[keep] (~63MB read + ~63MB
     write) sharded across 8 NeuronCores with the custom GPSIMD dma_gather
     instruction; each core gathers an equal slice of the keep list from a
     contiguous window of x and writes its contiguous output block.
  3. Host: assemble (x[keep], new_edge_index, batch[keep]).
"""

import numpy as np
from contextlib import ExitStack

N_CORES = 8
GATHER_CHUNK = 1024  # idxs per dma_gather: 1024/16+1=65 of 128 ring entries


# ---------------------------------------------------------------- host part


def _collapse_host(x, edge_index, target_edge_count, batch):
    """Faithful, fast port of the reference _collapse. Returns (x_mask, new_ei)."""
    import itertools

    N = x.shape[0]
    # exactly as reference: numpy float32 pairwise row sum of squares
    mags = (x * x).sum(axis=1)

    src = edge_index[0].tolist()
    dst = edge_index[1].tolist()
    neigh = [[] for _ in range(N)]
    for f, t in zip(src, dst):
        neigh[f].append(t)

    collapse_mask = bytearray(b"\x01") * N
    x_mask = bytearray(b"\x01") * N

    batch_np = np.asarray(batch)
    n_batches = int(batch_np.max()) + 1
    for batch_id in range(n_batches):
        node_ids = np.nonzero(batch_np == batch_id)[0]
        num_nodes = int(node_ids.shape[0])
        m = mags[node_ids]
        # heap pop order == ascending (mag, node_id); stable sort breaks
        # float ties by node id exactly like the reference's [mag, id] lists
        order = node_ids[np.argsort(m, kind="stable")].tolist()
        ptr = 0
        n_order = len(order)
        while num_nodes > target_edge_count:
            if ptr >= n_order:
                break
            node_id = order[ptr]
            ptr += 1
            if not collapse_mask[node_id]:
                continue
            collapse_mask[node_id] = 0
            L = neigh[node_id]
            num_nodes -= 1
            newL = []
            Lset = set(L)
            Lset.add(node_id)
            append = newL.append
            for nb in L:
                x_mask[nb] = 0
                collapse_mask[nb] = 0
                num_nodes -= 1
                for nbnb in neigh[nb]:
                    if nbnb in Lset:
                        continue
                    append(nbnb)
                    nl = neigh[nbnb]
                    nl2 = [n for n in nl if n != nb]
                    nl2.append(node_id)
                    neigh[nbnb] = nl2
            neigh[node_id] = newL

    x_mask_np = np.frombuffer(bytes(x_mask), dtype=np.uint8).astype(bool)
    new_to_old = np.nonzero(x_mask_np)[0]
    K = new_to_old.shape[0]
    old_to_new = -np.ones(N, dtype=np.int64)
    old_to_new[new_to_old] = np.arange(K)

    lengths = np.empty(K, dtype=np.int64)
    cols_chunks = []
    for i, old_id in enumerate(new_to_old.tolist()):
        nl = neigh[old_id]
        lengths[i] = len(nl)
        cols_chunks.append(nl)
    cols_flat = np.fromiter(
        itertools.chain.from_iterable(cols_chunks),
        dtype=np.int64,
        count=int(lengths.sum()),
    )
    rows = np.repeat(np.arange(K, dtype=np.int64), lengths)
    cols = old_to_new[cols_flat]
    new_ei = np.stack([rows, cols])
    return x_mask_np, new_ei


# -------------------------------------------------------------- device part


def _build_gather_nc(shard_rows: int, cap: int):
    """out[j,:] = xshard[idx-position-j,:]. dma_gather puts gather position
    i at SBUF[partition i%128, tile i//128]; we set idx position i=t*128+p
    to keep_rel[p*T+t] so SBUF[p, tile t] holds output row p*T+t, making the
    writeback a plain dense copy with per-partition contiguous DRAM."""
    import concourse.bacc as bacc
    import concourse.mybir as mybir
    from concourse.library_config import mlp as _mlp_lib

    chunk = GATHER_CHUNK
    assert cap % chunk == 0 and chunk % 128 == 0
    assert shard_rows <= 32767
    nchunks = cap // chunk
    nc = bacc.Bacc("TRN2")
    x = nc.dram_tensor(
        "xshard", [shard_rows, 128], mybir.dt.float32, kind="ExternalInput"
    )
    idx = nc.dram_tensor("idx", [128, cap // 16], mybir.dt.int16, kind="ExternalInput")
    out = nc.dram_tensor("out", [cap, 128], mybir.dt.float32, kind="ExternalOutput")
    out_v = out[:].rearrange("(p t) e -> p (t e)", p=128)  # [128, T*128]

    with (
        nc.sbuf_tensor("idx_sb", [128, cap // 16], mybir.dt.int16) as idx_sb,
        nc.sbuf_tensor("stage", [128, cap], mybir.dt.float32) as stage,
        nc.semaphore("isem") as isem,
        nc.semaphore("wsem") as wsem,
        ExitStack() as stack,
        nc.Block() as block,
    ):
        csem = [
            stack.enter_context(nc.semaphore(f"csem{c}")) for c in range(nchunks)
        ]  # noqa: ANT232

        @block.gpsimd
        def _(g):
            g.load_library(_mlp_lib)
            g.dma_start(idx_sb[:], idx[:]).then_inc(isem, 16)
            g.wait_ge(isem, 16)
            for c in range(nchunks):
                g.dma_gather(
                    out_ap=stage[:, c * chunk : (c + 1) * chunk].rearrange(
                        "p (t e) -> p t e", e=128
                    ),
                    in_ap=x[:],
                    idxs_ap=idx_sb[:, c * (chunk // 16) : (c + 1) * (chunk // 16)],
                    num_idxs=chunk,
                    num_idxs_reg=chunk,
                    elem_size=128,
                ).then_inc(csem[c], 16)

        @block.sync
        def _(s):
            for c in range(nchunks):
                s.wait_ge(csem[c], 16)
                s.dma_start(
                    out_v[:, c * chunk : (c + 1) * chunk],
                    stage[:, c * chunk : (c + 1) * chunk],
                ).then_inc(wsem, 16)
            s.wait_ge(wsem, 16 * nchunks)

    nc.compile()
    return nc


def _pack_idx(rel_idx: np.ndarray, cap: int) -> np.ndarray:
    """Pad with the last index to cap, permute so gather position t*128+p
    carries keep_rel[p*T+t], wrap into [16, cap/16] (position i at
    [i%16, i//16]) and replicate across the 8 GPSIMD partition groups."""
    n = rel_idx.shape[0]
    assert 0 < n <= cap
    T = cap // 128
    full = np.empty(cap, dtype=np.int16)
    full[:n] = rel_idx.astype(np.int16)
    full[n:] = rel_idx[-1]
    pos_val = full.reshape(128, T).T.ravel()
    wrapped = pos_val.reshape(cap // 16, 16).T
    return np.tile(wrapped, (8, 1)).copy()


_nc_cache = {}


def _device_gather(x: np.ndarray, keep: np.ndarray) -> np.ndarray:
    """x[keep] computed on the 8 NeuronCores (equal keep-slices, each core
    reads a contiguous window of x). Falls back to host numpy if the window
    of any slice exceeds the int16-indexable 32767 rows."""
    from concourse.bass_utils import run_bass_kernel_spmd

    N = x.shape[0]
    K = keep.shape[0]
    if K == 0:
        return np.empty((0, x.shape[1]), dtype=x.dtype)
    bounds = [(i * K) // N_CORES for i in range(N_CORES + 1)]
    chunks = [keep[bounds[i] : bounds[i + 1]] for i in range(N_CORES)]
    spans = [int(c[-1]) - int(c[0]) + 1 if len(c) else 1 for c in chunks]
    shard_rows = -(-max(spans) // 128) * 128
    if shard_rows > 32767 or any(len(c) == 0 for c in chunks):
        return x[keep]
    cap = -(-max(len(c) for c in chunks) // GATHER_CHUNK) * GATHER_CHUNK

    key = (shard_rows, cap)
    if key not in _nc_cache:
        _nc_cache[key] = _build_gather_nc(shard_rows, cap)
    nc = _nc_cache[key]

    in_maps = []
    for c in chunks:
        base = min(int(c[0]), N - shard_rows)
        base = max(base, 0)
        in_maps.append(
            {
                "xshard": x[base : base + shard_rows],
                "idx": _pack_idx(np.asarray(c) - base, cap),
            }
        )
    res = run_bass_kernel_spmd(nc, in_maps, core_ids=list(range(N_CORES)))
    return np.concatenate(
        [res.results[i]["out"][: len(chunks[i])] for i in range(N_CORES)]
    )


# ------------------------------------------------------------------- kernel


def kernel(x, edge_index, target_edge_count, batch, vertices, edges):
    x = np.asarray(x)
    ei = np.asarray(edge_index)
    batch_np = np.asarray(batch)
    tec = int(np.asarray(target_edge_count))

    x_mask, new_ei = _collapse_host(x, ei, tec, batch_np)
    keep = np.nonzero(x_mask)[0]

    x_keep = _device_gather(x, keep)
    out_ei = new_ei.astype(ei.dtype)
    batch_keep = batch_np[keep]
    return x_keep, out_ei, batch_keep


# revision 2
# speedup vs baseline: 1.0152x; 1.0152x over previous
"""CollapsePool kernel for Trainium2 (8 NeuronCores).

Structure:
  1. Host: exact port of the reference heap-driven greedy collapse.
     The reference builds its heap once (no pushes), so pop order is exactly
     ascending (mag, node_id): a stable argsort over per-graph mags
     reproduces it bit-for-bit, including duplicate-neighbour semantics.
  2. Device (the memory-bound bulk): gather x[keep] (~63MB read + ~63MB
     write) sharded across 8 NeuronCores with the custom GPSIMD dma_gather
     instruction; each core gathers an equal slice of the keep list from a
     contiguous window of x and writes its contiguous output block.
  3. Host: assemble (x[keep], new_edge_index, batch[keep]).
"""

import numpy as np
from contextlib import ExitStack

N_CORES = 8
# idxs per dma_gather instruction. Ring budget: n/16+1 descriptors of the
# 128-entry SWDGE ring (1920 crashes HW; 1024 and 512 validated). 512 sims
# fastest: smaller tail exposure, gen still hidden under the gather DMA.
GATHER_CHUNK = 512


# ---------------------------------------------------------------- host part


def _collapse_host(x, edge_index, target_edge_count, batch):
    """Faithful, fast port of the reference _collapse. Returns (x_mask, new_ei)."""
    import itertools

    N = x.shape[0]
    # exactly as reference: numpy float32 pairwise row sum of squares
    mags = (x * x).sum(axis=1)

    src = edge_index[0].tolist()
    dst = edge_index[1].tolist()
    neigh = [[] for _ in range(N)]
    for f, t in zip(src, dst):
        neigh[f].append(t)

    collapse_mask = bytearray(b"\x01") * N
    x_mask = bytearray(b"\x01") * N

    batch_np = np.asarray(batch)
    n_batches = int(batch_np.max()) + 1
    for batch_id in range(n_batches):
        node_ids = np.nonzero(batch_np == batch_id)[0]
        num_nodes = int(node_ids.shape[0])
        m = mags[node_ids]
        # heap pop order == ascending (mag, node_id); stable sort breaks
        # float ties by node id exactly like the reference's [mag, id] lists
        order = node_ids[np.argsort(m, kind="stable")].tolist()
        ptr = 0
        n_order = len(order)
        while num_nodes > target_edge_count:
            if ptr >= n_order:
                break
            node_id = order[ptr]
            ptr += 1
            if not collapse_mask[node_id]:
                continue
            collapse_mask[node_id] = 0
            L = neigh[node_id]
            num_nodes -= 1
            newL = []
            Lset = set(L)
            Lset.add(node_id)
            append = newL.append
            for nb in L:
                x_mask[nb] = 0
                collapse_mask[nb] = 0
                num_nodes -= 1
                for nbnb in neigh[nb]:
                    if nbnb in Lset:
                        continue
                    append(nbnb)
                    nl = neigh[nbnb]
                    nl2 = [n for n in nl if n != nb]
                    nl2.append(node_id)
                    neigh[nbnb] = nl2
            neigh[node_id] = newL

    x_mask_np = np.frombuffer(bytes(x_mask), dtype=np.uint8).astype(bool)
    new_to_old = np.nonzero(x_mask_np)[0]
    K = new_to_old.shape[0]
    old_to_new = -np.ones(N, dtype=np.int64)
    old_to_new[new_to_old] = np.arange(K)

    lengths = np.empty(K, dtype=np.int64)
    cols_chunks = []
    for i, old_id in enumerate(new_to_old.tolist()):
        nl = neigh[old_id]
        lengths[i] = len(nl)
        cols_chunks.append(nl)
    cols_flat = np.fromiter(
        itertools.chain.from_iterable(cols_chunks),
        dtype=np.int64,
        count=int(lengths.sum()),
    )
    rows = np.repeat(np.arange(K, dtype=np.int64), lengths)
    cols = old_to_new[cols_flat]
    new_ei = np.stack([rows, cols])
    return x_mask_np, new_ei


# -------------------------------------------------------------- device part


def _build_gather_nc(shard_rows: int, cap: int):
    """out[j,:] = xshard[idx-position-j,:]. dma_gather puts gather position
    i at SBUF[partition i%128, tile i//128]; we set idx position i=t*128+p
    to keep_rel[p*T+t] so SBUF[p, tile t] holds output row p*T+t, making the
    writeback a plain dense copy with per-partition contiguous DRAM."""
    import concourse.bacc as bacc
    import concourse.mybir as mybir
    from concourse.library_config import mlp as _mlp_lib

    chunk = GATHER_CHUNK
    assert cap % chunk == 0 and chunk % 128 == 0
    assert shard_rows <= 32767
    nchunks = cap // chunk
    nc = bacc.Bacc("TRN2")
    x = nc.dram_tensor(
        "xshard", [shard_rows, 128], mybir.dt.float32, kind="ExternalInput"
    )
    idx = nc.dram_tensor("idx", [128, cap // 16], mybir.dt.int16, kind="ExternalInput")
    out = nc.dram_tensor("out", [cap, 128], mybir.dt.float32, kind="ExternalOutput")
    out_v = out[:].rearrange("(p t) e -> p (t e)", p=128)  # [128, T*128]

    with (
        nc.sbuf_tensor("idx_sb", [128, cap // 16], mybir.dt.int16) as idx_sb,
        nc.sbuf_tensor("stage", [128, cap], mybir.dt.float32) as stage,
        nc.semaphore("isem") as isem,
        nc.semaphore("wsem") as wsem,
        ExitStack() as stack,
        nc.Block() as block,
    ):
        csem = [
            stack.enter_context(nc.semaphore(f"csem{c}")) for c in range(nchunks)
        ]  # noqa: ANT232

        @block.gpsimd
        def _(g):
            g.load_library(_mlp_lib)
            g.dma_start(idx_sb[:], idx[:]).then_inc(isem, 16)
            g.wait_ge(isem, 16)
            for c in range(nchunks):
                g.dma_gather(
                    out_ap=stage[:, c * chunk : (c + 1) * chunk].rearrange(
                        "p (t e) -> p t e", e=128
                    ),
                    in_ap=x[:],
                    idxs_ap=idx_sb[:, c * (chunk // 16) : (c + 1) * (chunk // 16)],
                    num_idxs=chunk,
                    num_idxs_reg=chunk,
                    elem_size=128,
                ).then_inc(csem[c], 16)

        @block.sync
        def _(s):
            for c in range(nchunks):
                s.wait_ge(csem[c], 16)
                s.dma_start(
                    out_v[:, c * chunk : (c + 1) * chunk],
                    stage[:, c * chunk : (c + 1) * chunk],
                ).then_inc(wsem, 16)
            s.wait_ge(wsem, 16 * nchunks)

    nc.compile()
    return nc


def _pack_idx(rel_idx: np.ndarray, cap: int) -> np.ndarray:
    """Pad with the last index to cap, permute so gather position t*128+p
    carries keep_rel[p*T+t], wrap into [16, cap/16] (position i at
    [i%16, i//16]) and replicate across the 8 GPSIMD partition groups."""
    n = rel_idx.shape[0]
    assert 0 < n <= cap
    T = cap // 128
    full = np.empty(cap, dtype=np.int16)
    full[:n] = rel_idx.astype(np.int16)
    full[n:] = rel_idx[-1]
    pos_val = full.reshape(128, T).T.ravel()
    wrapped = pos_val.reshape(cap // 16, 16).T
    return np.tile(wrapped, (8, 1)).copy()


_nc_cache = {}


def _device_gather(x: np.ndarray, keep: np.ndarray) -> np.ndarray:
    """x[keep] computed on the 8 NeuronCores (equal keep-slices, each core
    reads a contiguous window of x). Falls back to host numpy if the window
    of any slice exceeds the int16-indexable 32767 rows."""
    from concourse.bass_utils import run_bass_kernel_spmd

    N = x.shape[0]
    K = keep.shape[0]
    if K == 0:
        return np.empty((0, x.shape[1]), dtype=x.dtype)
    bounds = [(i * K) // N_CORES for i in range(N_CORES + 1)]
    chunks = [keep[bounds[i] : bounds[i + 1]] for i in range(N_CORES)]
    spans = [int(c[-1]) - int(c[0]) + 1 if len(c) else 1 for c in chunks]
    shard_rows = -(-max(spans) // 128) * 128
    if shard_rows > 32767 or any(len(c) == 0 for c in chunks):
        return x[keep]
    cap = -(-max(len(c) for c in chunks) // GATHER_CHUNK) * GATHER_CHUNK

    key = (shard_rows, cap)
    if key not in _nc_cache:
        _nc_cache[key] = _build_gather_nc(shard_rows, cap)
    nc = _nc_cache[key]

    in_maps = []
    for c in chunks:
        base = min(int(c[0]), N - shard_rows)
        base = max(base, 0)
        in_maps.append(
            {
                "xshard": x[base : base + shard_rows],
                "idx": _pack_idx(np.asarray(c) - base, cap),
            }
        )
    res = run_bass_kernel_spmd(nc, in_maps, core_ids=list(range(N_CORES)))
    return np.concatenate(
        [res.results[i]["out"][: len(chunks[i])] for i in range(N_CORES)]
    )


# ------------------------------------------------------------------- kernel


def kernel(x, edge_index, target_edge_count, batch, vertices, edges):
    x = np.asarray(x)
    ei = np.asarray(edge_index)
    batch_np = np.asarray(batch)
    tec = int(np.asarray(target_edge_count))

    x_mask, new_ei = _collapse_host(x, ei, tec, batch_np)
    keep = np.nonzero(x_mask)[0]

    x_keep = _device_gather(x, keep)
    out_ei = new_ei.astype(ei.dtype)
    batch_keep = batch_np[keep]
    return x_keep, out_ei, batch_keep
